# revision 9
# baseline (speedup 1.0000x reference)
"""Trainium2 Bass kernel: batched RK4 integration of a tiny MLP neural ODE.

Math reformulation (validated to ~1e-8 rel err vs the jax reference):
  The MLP field is f(y) = W2 tanh(W1 y + b1) + b2 with y in R^2, hidden 16.
  Work in u = W1 y + b1 (hidden pre-activation, R^16). With A = W1 W2,
  c = W1 b2, every RK4 stage becomes:
     th_j = tanh(arg_j);  arg_{j+1} = u + s_j * (A th_j + c)
     u'   = u + (h/6)(v1 + 2 v2 + 2 v3 + v4),  v_j = A th_j + c
     y'   = y + (h/6) W2 (th1 + 2 th2 + 2 th3 + th4) + h b2
  so the only nonlinearity is tanh on 16-vectors and every matmul is a
  fixed 16x16 (or 16x2) contraction.

Layout: each 18-partition block holds one particle's state: rows 0..15 = u,
rows 16..17 = y.  7 blocks -> 126 partitions.  Columns = independent
particles.  All per-step matmuls are 126x126 block-diagonal fp16
stationaries on the PE (1 cycle/column), tanh runs on ACT straight out of
PSUM with per-partition bias, and one fused DVE scalar_tensor_tensor
applies X' = (psumC + bias) + X, advancing u AND y together in fp32.
y rows are DMA'd to HBM from SBUF each step; y_0 is the input itself
(host side).  fp16 only ever touches matmul operands (error ~1e-4: tanh
saturation suppresses the u rounding error exactly where |u| is large).
"""

import numpy as np

# ---- problem constants (hardcoded per the harness contract) ----
N_TOTAL = 1048576
H = 16
NCORES = 8
NPC = N_TOTAL // NCORES      # particles per core
NSTEP = 10

# ---- packing configuration ----
F = 512                      # columns per chunk tile
BLK = 7                      # particle blocks stacked in partition dim
RPB = 18                     # rows per block: 16 u + 2 y
PART = BLK * RPB             # 126 active partitions
UROWS = BLK * H              # 112: u rows live in partitions 0..111
CHUNK = BLK * F              # particles per chunk
NCHUNK = -(-NPC // CHUNK)    # chunks per core (ceil)
NPC_PAD = NCHUNK * CHUNK
MM_N = 512                   # matmul free dim per instruction (one PSUM bank)
NSTAT = 1 + 4 * NSTEP        # identity, then per-step A2h/A4h/G16/G13
NBIAS = 4 * NSTEP + 1

# pool sizing knobs
STAGE_BUFS = 3
C_BUFS = 4
X_BUFS = 2 * C_BUFS
TH_BUFS = 4 * C_BUFS

_I_STAT = 0


def _t_seq():
    return np.concatenate(
        [np.arange(0.0, 0.1, 0.01, dtype=np.float32),
         np.array([0.1], dtype=np.float32)])


def _build_consts(W1, b1, W2, b2):
    """Stationaries: fp16 [NSTAT,128,128] + fp32 Winit [128,128];
    bias table fp32 [128, NBIAS]."""
    f64 = np.float64
    t = _t_seq()
    hs = np.diff(t).astype(f64)  # exact per-step fp32 step sizes, as f64
    A = W1.astype(f64) @ W2.astype(f64)          # [16,16]
    c = W1.astype(f64) @ b2.astype(f64)          # [16]

    # partition layout: u rows of block b at 16b..16b+15, y rows at 112+2b..+1
    def U(b):
        return slice(16 * b, 16 * b + H)

    def Y(b):
        return slice(UROWS + 2 * b, UROWS + 2 * b + 2)

    def ustat(M):
        # lhsT: th rows -> u rows via M [16,16]  (lhsT[k, m] = M[m, k])
        out = np.zeros((128, 128), f64)
        for b in range(BLK):
            out[U(b), U(b)] = M.T
        return out

    def gstat(hA, hW2):
        # lhsT: th rows -> u-inc rows (hA) and y-inc rows (hW2)
        out = np.zeros((128, 128), f64)
        for b in range(BLK):
            out[U(b), U(b)] = hA.T
            out[U(b), Y(b)] = hW2.T
        return out

    stats = np.zeros((NSTAT, 128, 128), f64)
    stats[_I_STAT] = ustat(np.eye(H))

    biases = np.zeros((128, NBIAS), f64)
    for n in range(NSTEP):
        h = hs[n]
        stats[1 + 4 * n + 0] = ustat(h / 2 * A)
        stats[1 + 4 * n + 1] = ustat(h * A)
        stats[1 + 4 * n + 2] = gstat(h / 6 * A, h / 6 * W2.astype(f64))
        stats[1 + 4 * n + 3] = gstat(h / 3 * A, h / 3 * W2.astype(f64))
        for b in range(BLK):
            biases[U(b), 4 * n + 0] = h / 2 * c
            biases[U(b), 4 * n + 1] = h * c
            biases[U(b), 4 * n + 2] = h * c
            biases[Y(b), 4 * n + 2] = h * b2.astype(f64)
    for b in range(BLK):
        biases[U(b), 4 * NSTEP] = b1.astype(f64)

    stats16 = np.ascontiguousarray(
        stats.transpose(1, 0, 2).reshape(128, NSTAT * 128)).astype(np.float16)

    # init: x rows (2b+s) -> u rows (W1) and y rows (identity)
    winit = np.zeros((128, 128), f64)
    for b in range(BLK):
        winit[2 * b:2 * b + 2, U(b)] = W1.astype(f64).T
        winit[2 * b:2 * b + 2, Y(b)] = np.eye(2)

    return stats16, winit.astype(np.float32), biases.astype(np.float32)


def _build_nc():
    import concourse.mybir as mybir
    from concourse import bacc
    from concourse.tile import TileContext

    f32 = mybir.dt.float32
    f16 = mybir.dt.float16
    TANH = mybir.ActivationFunctionType.Tanh
    IDENT = mybir.ActivationFunctionType.Identity
    ADD = mybir.AluOpType.add

    nc = bacc.Bacc(None, target_bir_lowering=False)

    x_in = nc.declare_dram_parameter("x_in", [NCHUNK, 2 * BLK, F], f32, isOutput=False)
    stats = nc.declare_dram_parameter("stats", [128, NSTAT * 128], f16, isOutput=False)
    winit = nc.declare_dram_parameter("winit", [128, 128], f32, isOutput=False)
    biases = nc.declare_dram_parameter("biases", [128, NBIAS], f32, isOutput=False)
    y_out = nc.declare_dram_parameter(
        "y_out", [NSTEP, NCHUNK, BLK * 2, F], f32, isOutput=True)

    NH = F // MM_N  # matmul instructions per logical matmul

    with TileContext(nc) as tc:
        with (
            tc.tile_pool(name="consts", bufs=1) as consts,
            tc.tile_pool(name="xin", bufs=4) as xin_pool,
            tc.tile_pool(name="xst", bufs=X_BUFS) as x_pool,
            tc.tile_pool(name="xh", bufs=X_BUFS) as xh_pool,
            tc.tile_pool(name="th", bufs=TH_BUFS) as th_pool,
            tc.tile_pool(name="stage", bufs=STAGE_BUFS, space="PSUM") as stage_pool,
            tc.tile_pool(name="acc", bufs=C_BUFS, space="PSUM") as c_pool,
        ):
            stats_t = consts.tile([128, NSTAT * 128], f16)
            nc.sync.dma_start(out=stats_t[:, :], in_=stats[:, :])
            winit_t = consts.tile([128, 128], f32)
            nc.sync.dma_start(out=winit_t[:, :], in_=winit[:, :])
            bias_t = consts.tile([128, NBIAS], f32)
            nc.sync.dma_start(out=bias_t[:, :], in_=biases[:, :])

            def lhsT(idx):
                return stats_t[0:PART, idx * 128: idx * 128 + PART]

            def bias_ap(col):
                return bias_t[0:PART, col:col + 1]

            def mm(ps, stat_idx, rhs, start, stop):
                for hh in range(NH):
                    cs = slice(hh * MM_N, (hh + 1) * MM_N)
                    nc.tensor.matmul(
                        out=ps[0:PART, cs],
                        lhsT=lhsT(stat_idx),
                        rhs=rhs[0:PART, cs],
                        start=start, stop=stop)

            for ch in range(NCHUNK):
                xin = xin_pool.tile([2 * BLK, F], f32, tag="xin")
                nc.sync.dma_start(out=xin[:, :], in_=x_in[ch])

                # X0: u rows = W1 x + b1, y rows = x  (full-precision matmul)
                ps0 = stage_pool.tile([PART, F], f32, tag="stage")
                for hh in range(NH):
                    cs = slice(hh * MM_N, (hh + 1) * MM_N)
                    nc.tensor.matmul(
                        out=ps0[0:PART, cs],
                        lhsT=winit_t[0:2 * BLK, 0:PART],
                        rhs=xin[0:2 * BLK, cs],
                        start=True, stop=True)
                X = x_pool.tile([PART, F], f32, tag="X")
                nc.scalar.activation(X[0:PART, :], ps0[0:PART, :], IDENT,
                                     bias=bias_ap(4 * NSTEP))

                for n in range(NSTEP):
                    s_a2, s_a4, s_g16, s_g13 = (1 + 4 * n, 2 + 4 * n,
                                                3 + 4 * n, 4 + 4 * n)
                    Xh = xh_pool.tile([PART, F], f16, tag="Xh")
                    nc.vector.tensor_copy(Xh[0:PART, :], X[0:PART, :])

                    th1 = th_pool.tile([PART, F], f16, tag="th")
                    nc.scalar.activation(th1[0:PART, :], X[0:PART, :], TANH)

                    pC = c_pool.tile([PART, F], f32, tag="acc")
                    mm(pC, s_g16, th1, start=True, stop=False)

                    psA = stage_pool.tile([PART, F], f32, tag="stage")
                    mm(psA, _I_STAT, Xh, start=True, stop=False)
                    mm(psA, s_a2, th1, start=False, stop=True)
                    th2 = th_pool.tile([PART, F], f16, tag="th")
                    nc.scalar.activation(th2[0:PART, :], psA[0:PART, :], TANH,
                                         bias=bias_ap(4 * n + 0))
                    mm(pC, s_g13, th2, start=False, stop=False)

                    psB = stage_pool.tile([PART, F], f32, tag="stage")
                    mm(psB, _I_STAT, Xh, start=True, stop=False)
                    mm(psB, s_a2, th2, start=False, stop=True)
                    th3 = th_pool.tile([PART, F], f16, tag="th")
                    nc.scalar.activation(th3[0:PART, :], psB[0:PART, :], TANH,
                                         bias=bias_ap(4 * n + 0))
                    mm(pC, s_g13, th3, start=False, stop=False)

                    psD = stage_pool.tile([PART, F], f32, tag="stage")
                    mm(psD, _I_STAT, Xh, start=True, stop=False)
                    mm(psD, s_a4, th3, start=False, stop=True)
                    th4 = th_pool.tile([PART, F], f16, tag="th")
                    nc.scalar.activation(th4[0:PART, :], psD[0:PART, :], TANH,
                                         bias=bias_ap(4 * n + 1))
                    mm(pC, s_g16, th4, start=False, stop=True)

                    Xn = x_pool.tile([PART, F], f32, tag="X")
                    nc.vector.scalar_tensor_tensor(
                        out=Xn[0:PART, :], in0=pC[0:PART, :],
                        scalar=bias_ap(4 * n + 2), in1=X[0:PART, :],
                        op0=ADD, op1=ADD)

                    nc.sync.dma_start(out=y_out[n, ch],
                                      in_=Xn[UROWS:PART, :])
                    X = Xn
    nc.finalize()
    return nc


_NC_CACHE = None


def _get_nc():
    global _NC_CACHE
    if _NC_CACHE is None:
        _NC_CACHE = _build_nc()
    return _NC_CACHE


def _pack_inputs(x, W1, b1, W2, b2):
    stats16, winit, biases = _build_consts(W1, b1, W2, b2)
    x = np.asarray(x, np.float32).reshape(-1, 2)
    in_maps = []
    for core in range(NCORES):
        xs = x[core * NPC:(core + 1) * NPC]
        xp = np.zeros((NPC_PAD, 2), np.float32)
        xp[:NPC] = xs
        # device layout [NCHUNK, BLK, 2, F]: particle = ch*CHUNK + blk*F + col
        xdev = np.ascontiguousarray(
            xp.reshape(NCHUNK, BLK, F, 2).transpose(0, 1, 3, 2)
        ).reshape(NCHUNK, 2 * BLK, F)
        in_maps.append({"x_in": xdev, "stats": stats16, "winit": winit,
                        "biases": biases})
    return in_maps


def _unpack_outputs(results, x):
    ys = []
    for core in range(NCORES):
        yd = results[core]["y_out"]  # [NSTEP, NCHUNK, BLK*2, F]
        yd = np.asarray(yd).reshape(NSTEP, NCHUNK, BLK, 2, F)
        yd = yd.transpose(0, 1, 2, 4, 3).reshape(NSTEP, NPC_PAD, 2)[:, :NPC]
        ys.append(yd)
    y_steps = np.concatenate(ys, axis=1)          # [NSTEP, N, 2]
    x = np.asarray(x, np.float32).reshape(1, -1, 2)
    return np.concatenate([x, y_steps], axis=0)   # [NSTEP+1, N, 2]


def run_device(in_maps, trace=False, **kw):
    from concourse import bass_utils
    nc = _get_nc()
    return bass_utils.run_bass_kernel_spmd(
        nc, in_maps, core_ids=list(range(NCORES)), trace=trace, **kw)


def kernel(x, W1, b1, W2, b2):
    in_maps = _pack_inputs(x, W1, b1, W2, b2)
    res = run_device(in_maps)
    y = _unpack_outputs(res.results, x)
    return _t_seq(), y
